# revision 9
# baseline (speedup 1.0000x reference)
"""Bahdanau attention kernel for 8 Trainium2 NeuronCores.

Computation (per batch row b):
    scores[t] = Wo . tanh(We @ enc[t,b] + (mean_L(hidden) @ Wh.T + bh + be))
    out[b]    = masked_softmax(scores, enc_len[b])

Sharding: data-parallel over batch (8 rows per core). The [H,H] projection
of the encoder states dominates (B*T rows x [512,512]); it runs as bf16
matmuls on the PE with fp32 PSUM accumulation. encoder_output is cast
fp32->bf16 during the DMA load (SWDGE), transposed on-chip with PE
transpose-mode into [h, t] layout, and contracted against We^T. tanh and
the per-(b,oc) bias add are fused into one ScalarE activation pass that
also reads PSUM directly. The Wo reduction is a K=o matmul producing
[1, t] scores per 512-token chunk; the masked softmax runs
partition-parallel over the 8 batch rows at the end.
"""

import numpy as np
import ml_dtypes

import concourse.bass as bass
import concourse.mybir as mybir
import concourse.tile as tile
from concourse.bass_utils import run_bass_kernel_spmd

BF16 = mybir.dt.bfloat16
F32 = mybir.dt.float32

L, B, T, H = 2, 64, 2048, 512
NCORE = 8
BL = B // NCORE  # batch rows per core
NCHUNK = T // 512  # 512-token chunks per row
NEG = -1e30
DEBUG_SCORES = False


def _split_multi_waits(nc):
    # This walrus build accepts only one sync-wait per instruction. Tile's
    # scheduler attaches one wait per outstanding proc (the exit drain can
    # carry many), so hoist extra waits onto single-wait NOP carriers
    # inserted just before the instruction on the same engine.
    for fn in nc.m.functions:
        for blk in fn.blocks:
            out = []
            changed = False
            for inst in blk.instructions:
                si = inst.sync_info
                waits = list(si.on_wait) if si is not None else []
                if len(waits) > 1:
                    changed = True
                    for k, w in enumerate(waits):
                        nop = mybir.InstNoOp(
                            name=f"{inst.name}-wc{k}", ins=[], outs=[]
                        )
                        nop.engine = inst.engine
                        nop.sync_info = mybir.SyncInfo(on_wait=[w], on_update=[])
                        out.append(nop)
                    inst.sync_info = mybir.SyncInfo(
                        on_wait=[], on_update=list(si.on_update)
                    )
                out.append(inst)
            if changed:
                blk.instructions = out
    return nc


def _build_program():
    nc = bass.Bass()
    enc = nc.dram_tensor("enc", [T, BL, H], F32, kind="ExternalInput")
    wet = nc.dram_tensor("wet", [4, 128, H], BF16, kind="ExternalInput")
    wo = nc.dram_tensor("wo", [128, 4], BF16, kind="ExternalInput")
    cvec = nc.dram_tensor("cvec", [128, BL * 4], F32, kind="ExternalInput")
    mneg = nc.dram_tensor("mneg", [BL, T], F32, kind="ExternalInput")
    iden = nc.dram_tensor("iden", [128, 128], BF16, kind="ExternalInput")
    wout = nc.dram_tensor("wout", [BL, T], F32, kind="ExternalOutput")
    if DEBUG_SCORES:
        sc_dram = nc.dram_tensor("sc_scratch", [BL, T], F32, kind="ExternalOutput")
    else:
        sc_dram = nc.dram_tensor("sc_scratch", [BL, T], F32)

    Tanh = mybir.ActivationFunctionType.Tanh
    Exp = mybir.ActivationFunctionType.Exp
    Copy = mybir.ActivationFunctionType.Copy

    with tile.TileContext(nc) as tc:
        with (
            tc.tile_pool(name="consts", bufs=1) as consts,
            tc.tile_pool(name="encp", bufs=2) as encp,
            tc.tile_pool(name="etp", bufs=3) as etp,
            tc.tile_pool(name="resp", bufs=8) as resp,
            tc.tile_pool(name="soft", bufs=1) as soft,
            tc.tile_pool(name="tps", bufs=2, space="PSUM") as tps,
            tc.tile_pool(name="mmp", bufs=3, space="PSUM") as mmp,
            tc.tile_pool(name="scp", bufs=1, space="PSUM") as scp,
        ):
            wet_sb = consts.tile([128, 4, H], BF16)
            nc.sync.dma_start(out=wet_sb, in_=wet[:].rearrange("c p o -> p c o"))
            wo_sb = consts.tile([128, 4], BF16)
            nc.sync.dma_start(out=wo_sb, in_=wo[:])
            cv_sb = consts.tile([128, BL * 4], F32)
            nc.sync.dma_start(out=cv_sb, in_=cvec[:])
            id_sb = consts.tile([128, 128], BF16)
            nc.sync.dma_start(out=id_sb, in_=iden[:])
            mneg_sb = consts.tile([BL, T], F32)
            nc.sync.dma_start(out=mneg_sb, in_=mneg[:])
            # per-(b,chunk) scores staged on partition 0 (compute engines
            # cannot write APs at unaligned base partitions), then bounced
            # through DRAM into [BL, T] partition-parallel layout for softmax
            scores_flat = consts.tile([1, BL * T], F32)
            scores_all = consts.tile([BL, T], F32)

            for b in range(BL):
                enc_nat = encp.tile([128, T // 128, H], BF16)
                # fp32 -> bf16 cast happens inside the SWDGE DMA
                nc.gpsimd.dma_start(
                    out=enc_nat,
                    in_=enc[:, b, :].rearrange("(n p) h -> p n h", p=128),
                )
                for ck in range(NCHUNK):
                    trans = tps.tile([128, 4, 512], BF16)  # [h, hc, t]
                    for ts in range(4):
                        n = ck * 4 + ts
                        for hc in range(4):
                            nc.tensor.transpose(
                                trans[:, hc, ts * 128 : (ts + 1) * 128],
                                enc_nat[:, n, hc * 128 : (hc + 1) * 128],
                                id_sb,
                            )
                    encT = etp.tile([128, 4, 512], BF16)
                    nc.vector.tensor_copy(encT, trans)
                    res_tiles = []
                    for oc in range(4):
                        matt = mmp.tile([128, 512], F32)
                        for hc in range(4):
                            nc.tensor.matmul(
                                matt,
                                wet_sb[:, hc, oc * 128 : (oc + 1) * 128],
                                encT[:, hc, :],
                                start=(hc == 0),
                                stop=(hc == 3),
                            )
                        res = resp.tile([128, 512], BF16)
                        nc.scalar.activation(
                            res, matt, Tanh, bias=cv_sb[:, b * 4 + oc : b * 4 + oc + 1]
                        )
                        res_tiles.append(res)
                    score = scp.tile([1, 512], F32)
                    for oc in range(4):
                        nc.tensor.matmul(
                            score,
                            wo_sb[:, oc : oc + 1],
                            res_tiles[oc],
                            start=(oc == 0),
                            stop=(oc == 3),
                        )
                    nc.scalar.activation(
                        scores_flat[0:1, b * T + ck * 512 : b * T + (ck + 1) * 512],
                        score,
                        Copy,
                    )

            nc.sync.dma_start(
                out=sc_dram[:],
                in_=scores_flat.rearrange("p (b t) -> p b t", b=BL),
            )
            nc.sync.dma_start(out=scores_all[0:BL, :], in_=sc_dram[:])

            # masked softmax over the full T axis, partition-parallel over b
            sm = soft.tile([BL, T], F32)
            nc.vector.tensor_add(sm, scores_all[0:BL, :], mneg_sb)
            ex = soft.tile([BL, T], F32)
            esum = soft.tile([BL, 1], F32)
            nc.scalar.activation(ex, sm, Exp, accum_out=esum)
            rec = soft.tile([BL, 1], F32)
            nc.vector.reciprocal(rec, esum)
            wt = soft.tile([BL, T], F32)
            nc.vector.tensor_scalar_mul(wt, ex, rec)
            nc.sync.dma_start(out=wout[:], in_=wt)

    return _split_multi_waits(nc)


_PROGRAM = None


def _program():
    global _PROGRAM
    if _PROGRAM is None:
        _PROGRAM = _build_program()
    return _PROGRAM


def kernel(hidden, encoder_output, enc_len, Wh, bh, We, be, Wo, bo):
    hidden = np.asarray(hidden, dtype=np.float32)
    encoder_output = np.asarray(encoder_output, dtype=np.float32)
    enc_len = np.asarray(enc_len, dtype=np.int32)
    Wh = np.asarray(Wh, dtype=np.float32)
    bh = np.asarray(bh, dtype=np.float32)
    We = np.asarray(We, dtype=np.float32)
    be = np.asarray(be, dtype=np.float32)
    Wo = np.asarray(Wo, dtype=np.float32)
    bo = np.asarray(bo, dtype=np.float32)

    # small host-side prep: bias vector c = mean_L(hidden) @ Wh.T + bh + be
    h = hidden.mean(axis=0, dtype=np.float64)  # [B, H]
    c = (h @ Wh.T.astype(np.float64) + bh + be).astype(np.float32)  # [B, H]

    wet_np = np.ascontiguousarray(We.T).reshape(4, 128, H).astype(ml_dtypes.bfloat16)
    wo_np = np.ascontiguousarray(Wo.reshape(4, 128).T).astype(ml_dtypes.bfloat16)
    iden_np = np.eye(128, dtype=ml_dtypes.bfloat16)
    maskneg = np.where(
        np.arange(T)[None, :] < enc_len[:, None], 0.0, NEG
    ).astype(np.float32)  # [B, T]

    in_maps = []
    for core in range(NCORE):
        bs = slice(core * BL, (core + 1) * BL)
        c_core = c[bs]  # [BL, H]
        cv = np.ascontiguousarray(
            c_core.reshape(BL, 4, 128).transpose(2, 0, 1).reshape(128, BL * 4)
        )
        in_maps.append(
            {
                "enc": np.ascontiguousarray(encoder_output[:, bs, :]),
                "wet": wet_np,
                "wo": wo_np,
                "cvec": cv,
                "mneg": np.ascontiguousarray(maskneg[bs]),
                "iden": iden_np,
            }
        )

    nc = _program()
    results = run_bass_kernel_spmd(nc, in_maps, list(range(NCORE))).results
    w = np.concatenate([results[core]["wout"] for core in range(NCORE)], axis=0)
    return w[:, :, None].astype(np.float32)
